# revision 28
# baseline (speedup 1.0000x reference)
"""Trainium2 Bass kernel for nn_MultiHeadAttention_446676599023.

Strategy (8 NeuronCores, SPMD, no collectives):
  core c -> batch b = c//2, head-group g = c%2 (heads 8g..8g+7, E-dims 512g..512g+512).

Math: reference computes attn_out = softmax(QK^T/sqrt(D)) @ V per head, projects with
Wo, takes mean over sequence, normalizes, subtracts text_array, then a tiny MLP.
mean_S commutes with the output projection, so each core only needs
  r_h[d] = sum_q softmax_row(q) @ V_h  summed over q   (shape [64] per head)
and the whole Wo/normalize/MLP tail runs on host on a [4,1024] tensor (exact algebra,
negligible FLOPs). Device work per core:
  - x^T and all projection weights ship as fp8 e4m3; Q/K/V projections run as
    fp8 DoubleRow matmuls (256-wide contraction per instruction -> half the
    accumulation passes of bf16). x^T loads in 16 column-chunk DMAs
    round-robined over the 3 DMA queues (~27GB/s each) so the first
    projections start ~8us in.
  - Q^T,K^T land in SBUF as fp8 e4m3 [d-part, seq-free] (ScalarE converts,
    bias fused); V as fp8 [seq-part, d-free] with a ones column per head
    (66-stride) so the EV matmul also emits the denominator Z as row 64.
    Keeping every matmul operand fp8 matters: the PE pays ~130ns per
    bf16<->fp8 datapath-mode switch, so the stream is kept single-mode.
  - scores^T[k,q] = plain fp8 matmul (contraction d=64) written as
    [128,2,512] f32 PSUM pairs spanning two banks, single-shot per half.
  - E = exp(scores/8): one paired op per [128,2,512] tile, split ~115:141
    between ScalarE (table exp -> f8e5) and DVE (Schraudolph int8 affine,
    bitcast f8e5, ~2-4% rel err, attenuated to ~1e-3 by the tail). With an
    all-ones mask the exp ops take no bias vector, enabling the paired form;
    a general-mask fallback path uses per-kt ops with mask-bias vectors.
    (Do NOT fold extra bias into EXB8: shifting the affine moves the int8
    NaN window from -10.4 sigma up into the score distribution's tail.)
  - P^T[d,q] (+ Z row) = DoubleRow matmul(lhsT=V_aug[k,2,65], rhs=E^T[k,2,q])
    accumulated over the 8 key-tile pairs. Heads run sequentially so PSUM
    fits exactly (3 score pairs 12KB/part + pacc 2KB + proj 2KB), and each
    head's EV burst + drain is deferred into the next head's window so the
    PE never barriers on the slowest exp.
  - finalize per (head, q-chunk): one drain to SBUF bf16, DMA ships it out.
Host does r[d] = sum_q P[d,q]/Z[q] (0.014%% of total FLOPs), then /S and the exact
Wo/normalize/MLP tail.
"""

import math
import os
import sys

import numpy as np

for _p in ("/opt/trn_rl_repo",):
    if _p not in sys.path and os.path.isdir(_p):
        sys.path.append(_p)

B, S, E, H = 4, 2048, 1024, 16
D = E // H            # 64 head dim
G = 2                 # head groups (tensor-parallel factor)
EG = E // G           # 512 dims per group
HG = H // G           # 8 heads per group
NCORES = 8
PART = 128
EP = 4                # fp8 contraction pair-tiles (256 e-dims each)
KT = S // PART        # 16 key tiles
JP = KT // 2          # 8 key-tile pairs
MT = EG // PART       # 4 head-pairs per group
QC = 4                # q chunks
QW = S // QC          # 512
NEG = -1.0e30

# Schraudolph fast-exp in fp8-e5m2 bit space: bitcast(int8(x*EXA8 + EXB8))
# ~= exp(x) as e5m2. For raw scores s in [-80, 80] the affine lands in
# [3, 117] -- always positive, never in the NaN encodings (>=124), and the
# masked bias saturates the int8 convert to -128 = -0.0.
EXA8 = 5.770780163555851   # 2^2 / ln(2)
EXB8 = 59.825              # 15 * 4 - 366393/2^21 (min max-rel-err bias)

_CACHE: dict = {}


def _build(maskfree: bool = True, repeat: int = 1):
    """Build the Bacc module (one SPMD program, same on all 8 cores)."""
    import concourse.bacc as bacc
    import concourse.mybir as mybir
    import concourse.tile as tile
    from contextlib import ExitStack

    f32 = mybir.dt.float32
    bf16 = mybir.dt.bfloat16
    f8 = mybir.dt.float8e4   # x / weights / V tiles (values ~N(0,1))
    f8e = mybir.dt.float8e5  # e tiles (exp up to e^9.8 needs e5m2 range)
    i8 = mybir.dt.int8
    PM = mybir.MatmulPerfMode
    AF = mybir.ActivationFunctionType
    ALU = mybir.AluOpType

    nc = bacc.Bacc("TRN2", target_bir_lowering=False, debug=False)
    x8T = nc.dram_tensor("x8T", [EP, PART, 2, S], f8, kind="ExternalInput").ap()
    wq8T = nc.dram_tensor(
        "wq8T", [MT, EP, PART, 2, PART], f8, kind="ExternalInput"
    ).ap()
    wk8T = nc.dram_tensor(
        "wk8T", [MT, EP, PART, 2, PART], f8, kind="ExternalInput"
    ).ap()
    wv8T = nc.dram_tensor("wv8T", [EP, PART, 2, EG], f8, kind="ExternalInput").ap()
    bqT = nc.dram_tensor("bqT", [PART, MT], f32, kind="ExternalInput").ap()
    bkT = nc.dram_tensor("bkT", [PART, MT], f32, kind="ExternalInput").ap()
    if not maskfree:
        mbT = nc.dram_tensor("mbT", [PART, KT], f32, kind="ExternalInput").ap()
        mbAT = nc.dram_tensor("mbAT", [PART, KT], f32, kind="ExternalInput").ap()
    resP = nc.dram_tensor(
        "resP", [repeat, HG, 65, S], bf16, kind="ExternalOutput"
    ).ap()

    with tile.TileContext(nc) as tc, ExitStack() as ctx:
        const_p = ctx.enter_context(tc.tile_pool(name="const", bufs=1))
        x8_p = ctx.enter_context(tc.tile_pool(name="x8", bufs=EP))
        wv_p = ctx.enter_context(tc.tile_pool(name="wv", bufs=EP))
        wqk_p = ctx.enter_context(tc.tile_pool(name="wqk", bufs=2 * EP * 2))
        qt_p = ctx.enter_context(tc.tile_pool(name="qt", bufs=2))
        kt_p = ctx.enter_context(tc.tile_pool(name="kt", bufs=2))
        v_p = ctx.enter_context(tc.tile_pool(name="v", bufs=JP))
        ea_p = ctx.enter_context(tc.tile_pool(name="ea", bufs=JP))
        eb_p = ctx.enter_context(tc.tile_pool(name="eb", bufs=JP))
        psb_p = ctx.enter_context(tc.tile_pool(name="psb", bufs=4))
        # PSUM (16KB/partition = 8 banks): 3 score pair-tiles [128,2,512] f32
        # (4KB each) + pacc [65,512] (2KB) + proj accumulator [128,512] (2KB).
        scp_ps = ctx.enter_context(tc.tile_pool(name="scps", bufs=3, space="PSUM"))
        p_ps = ctx.enter_context(tc.tile_pool(name="pps", bufs=1, space="PSUM"))
        qk_ps = ctx.enter_context(tc.tile_pool(name="qkps", bufs=1, space="PSUM"))

        # DMA queue rotation for outputs (gpsimd is idle after the prologue;
        # sync carries the weight stream early on)
        out_qs = (nc.gpsimd, nc.sync)

        for rep in range(repeat):
            # ---- input DMAs. Per-queue DMA bandwidth is ~27GB/s, so the
            # layout matters: p=0 weights first on sync (they gate the first
            # projection mms), then x8 in column chunks round-robined across
            # all three queues, chunk-major so early q-chunks land first ----
            bq = const_p.tile([PART, MT], f32, tag="bq")
            nc.gpsimd.dma_start(bq[:], bqT[:])
            bk = const_p.tile([PART, MT], f32, tag="bk")
            nc.gpsimd.dma_start(bk[:], bkT[:])
            if not maskfree:
                mb = const_p.tile([PART, KT], f32, tag="mb")
                nc.gpsimd.dma_start(mb[:], mbT[:])
                mb8 = const_p.tile([PART, KT], f32, tag="mb8")
                nc.gpsimd.dma_start(mb8[:], mbAT[:])
            wk8 = {}
            wq8 = {}

            def emit_wqk_dma(p, q=None):
                for wT, store in ((wk8T, wk8), (wq8T, wq8)):
                    tiles = []
                    for j in range(EP):
                        t = wqk_p.tile([PART, 2, PART], f8, tag="wqk")
                        (q or nc.sync).dma_start(t[:], wT[p, j])
                        tiles.append(t)
                    store[p] = tiles

            emit_wqk_dma(0)
            x8 = [
                x8_p.tile([PART, 2, S], f8, tag="x8", name=f"x8_{j}")
                for j in range(EP)
            ]
            x8_qs = (nc.gpsimd, nc.scalar, nc.sync)
            qi = 0
            for c in range(QC):
                for j in range(EP):
                    x8_qs[qi % 3].dma_start(
                        x8[j][:][:, :, c * QW : (c + 1) * QW],
                        x8T[j, :, :, c * QW : (c + 1) * QW],
                    )
                    qi += 1
            wv8 = []
            for j in range(EP):
                t = wv_p.tile([PART, 2, EG], f8, tag="wv")
                nc.gpsimd.dma_start(t[:], wv8T[j])
                wv8.append(t)
            for p in range(1, MT):
                emit_wqk_dma(p, (nc.scalar, nc.gpsimd, nc.sync)[p - 1])

            # ---- emission helpers ----
            v_sb = [None] * JP  # fp8 pair tiles [128, 2, HG*66]

            def emit_vproj(ks):
                j2, sub = ks // 2, ks % 2
                if sub == 0:
                    v_sb[j2] = v_p.tile(
                        [PART, 2, HG * 66], f8, tag="v", name=f"vp{j2}"
                    )
                v3 = v_sb[j2][:][:, sub, :].rearrange("p (h c) -> p h c", c=66)
                nc.vector.memset(v3[:, :, 64:66], 1.0)
                ps = qk_ps.tile([PART, EG], f32, tag="qkp", name="vps")
                for j in range(EP):
                    nc.tensor.matmul(
                        ps[:],
                        lhsT=x8[j][:][:, :, ks * PART : (ks + 1) * PART],
                        rhs=wv8[j][:],
                        start=(j == 0),
                        stop=(j == EP - 1),
                        perf_mode=PM.DoubleRow,
                    )
                # DVE cannot narrow f32->fp8; ScalarE can
                nc.scalar.copy(
                    v3[:, :, 0:64],
                    ps[:].rearrange("p (h c) -> p h c", c=64),
                )

            qkmats = {}

            def emit_qkproj_group(p, which, qc2):
                # which: 0 = K, 1 = Q; qc2: chunk of S (columns of K^T/Q^T)
                if (p, 0) not in qkmats and which == 0 and qc2 == 0:
                    qkmats[(p, 0)] = kt_p.tile(
                        [PART, S], f8, tag="kt", name=f"kt{p}"
                    )
                    qkmats[(p, 1)] = qt_p.tile(
                        [PART, S], f8, tag="qt", name=f"qt{p}"
                    )
                dst = qkmats[(p, which)]
                bias = (bk, bq)[which]
                wtiles = (wk8, wq8)[which][p]
                ps = qk_ps.tile([PART, QW], f32, tag="qkp", name="qkps")
                for j in range(EP):
                    nc.tensor.matmul(
                        ps[:],
                        lhsT=wtiles[j][:],
                        rhs=x8[j][:][:, :, qc2 * QW : (qc2 + 1) * QW],
                        start=(j == 0),
                        stop=(j == EP - 1),
                        perf_mode=PM.DoubleRow,
                    )
                # fp8 narrowing must go through ScalarE; bias rides along
                nc.scalar.activation(
                    dst[:, qc2 * QW : (qc2 + 1) * QW],
                    ps[:],
                    AF.Identity,
                    bias=bias[:, p : p + 1],
                )

            # ---- prologue: just enough for (p0, qc0, h0) to start, then V
            # and the rest stream in as fillers inside the attention loop ----
            for qc2 in range(QC):
                emit_qkproj_group(0, 0, qc2)   # K p0, full S
            emit_qkproj_group(0, 1, 0)         # Q p0, qc0
            emit_vproj(0)
            emit_vproj(1)

            # filler schedule: list of emit-closures per (p, qc, hl)
            def fillers_for(p, qc, hl):
                if p == 0:
                    if qc == 0 and hl == 0:
                        return [(emit_vproj, ks) for ks in range(2, 16)]
                    if qc == 0 and hl == 1:
                        return [(emit_qkproj_group, 0, 1, 1)]
                    if qc == 1:
                        return [(emit_qkproj_group, 0, 1, 2 + hl)]
                    # qc 2..3: next head-pair's K (4 chunks) + Q (4 chunks)
                    i0 = ((qc - 2) * 2 + hl) * 2
                    out = []
                    for i in (i0, i0 + 1):
                        which, c = (0, i) if i < 4 else (1, i - 4)
                        out.append((emit_qkproj_group, 1, which, c))
                    return out
                if p < MT - 1:
                    i = (qc * 2 + hl)
                    which, c = (0, i) if i < 4 else (1, i - 4)
                    return [(emit_qkproj_group, p + 1, which, c)]
                return []

            # ---- attention: heads sequential, kt pairs with 2-ahead
            # score pipelining; exp split ~115:141 ScalarE:DVE. Each head's
            # EV burst + drain is deferred into the NEXT head's window (after
            # its first scores) so the PE never barriers on the slowest exp.
            gidx = 0
            out_qi = 0
            pending = []

            def flush_pending():
                nonlocal out_qi
                if not pending:
                    return
                evs2, hcol2, p2, hl2, qc2 = pending.pop()
                pacc = p_ps.tile([65, QW], f32, tag="pp", name="pacc")
                for i, (jp2, et2, ia2) in enumerate(evs2):
                    nc.tensor.matmul(
                        pacc[:],
                        lhsT=v_sb[jp2][:][:, 0:2, hcol2 : hcol2 + 65],
                        rhs=et2[:] if ia2 else et2[:].bitcast(f8e),
                        start=(i == 0),
                        stop=(i == JP - 1),
                        perf_mode=PM.DoubleRow,
                    )
                psb = psb_p.tile([65, QW], bf16, tag="psb")
                nc.scalar.copy(psb[:], pacc[:])
                out_qs[out_qi % len(out_qs)].dma_start(
                    resP[rep, 2 * p2 + hl2, :, qc2 * QW : (qc2 + 1) * QW],
                    psb[:],
                )
                out_qi += 1

            for p in range(MT):
                for qc in range(QC):
                    for hl in (0, 1):
                        kt_m = qkmats[(p, 0)]
                        qt_m = qkmats[(p, 1)]
                        fills = list(fillers_for(p, qc, hl))
                        r0 = hl * 64
                        hcol = 66 * (2 * p + hl)
                        scp = [None] * JP

                        def emit_scores(jp):
                            t = scp_ps.tile(
                                [PART, 2, QW], f32, tag="sc", name=f"sc{jp}"
                            )
                            for half in (0, 1):
                                kt = 2 * jp + half
                                nc.tensor.matmul(
                                    t[:][:, half, :],
                                    lhsT=kt_m[
                                        r0 : r0 + 64, kt * PART : (kt + 1) * PART
                                    ],
                                    rhs=qt_m[r0 : r0 + 64, qc * QW : (qc + 1) * QW],
                                )
                            scp[jp] = t

                        emit_scores(0)
                        emit_scores(1)
                        flush_pending()
                        evs = []  # (jp, e-tile, is_act) -- EVs issued in the
                        # next window via flush_pending
                        for jp in range(JP):
                            if jp + 2 < JP:
                                emit_scores(jp + 2)
                            if fills:
                                # spread fillers evenly over remaining slots
                                k = -(-len(fills) // (JP - jp))
                                for _ in range(min(k, len(fills))):
                                    f = fills.pop(0)
                                    f[0](*f[1:])
                            use_act = ((gidx * 115) % 256) < 115
                            gidx += 1
                            if use_act:
                                et = ea_p.tile([PART, 2, QW], f8e, tag="et")
                                if maskfree:
                                    nc.scalar.activation(
                                        et[:],
                                        scp[jp][:],
                                        AF.Exp,
                                        scale=1.0 / math.sqrt(D),
                                    )
                                else:
                                    for half in (0, 1):
                                        kt = 2 * jp + half
                                        nc.scalar.activation(
                                            et[:][:, half, :],
                                            scp[jp][:][:, half, :],
                                            AF.Exp,
                                            bias=mb[:, kt : kt + 1],
                                            scale=1.0 / math.sqrt(D),
                                        )
                            else:
                                # Schraudolph e5m2 (safe affine range), same
                                # e^-2 bias folded into the constant
                                et = eb_p.tile([PART, 2, QW], i8, tag="ei")
                                if maskfree:
                                    nc.vector.tensor_scalar(
                                        et[:],
                                        scp[jp][:],
                                        EXA8 / 8.0,
                                        EXB8,
                                        ALU.mult,
                                        ALU.add,
                                    )
                                else:
                                    for half in (0, 1):
                                        kt = 2 * jp + half
                                        nc.vector.tensor_scalar(
                                            et[:][:, half, :],
                                            scp[jp][:][:, half, :],
                                            EXA8 / 8.0,
                                            mb8[:, kt : kt + 1],
                                            ALU.mult,
                                            ALU.add,
                                        )
                            evs.append((jp, et, use_act))
                        pending.append((evs, hcol, p, hl, qc))
            flush_pending()

    nc.compile()
    return nc


def get_nc(maskfree: bool = True, repeat: int = 1):
    key = ("nc", maskfree, repeat)
    if key not in _CACHE:
        _CACHE[key] = _build(maskfree, repeat)
    return _CACHE[key]


def make_in_maps(x, mask, Wq, bq, Wk, bk, Wv):
    """Per-core input dict (core c -> batch c//2, head-group c%2)."""
    import ml_dtypes

    f8 = ml_dtypes.float8_e4m3
    x = np.asarray(x, np.float32)
    mask = np.asarray(mask)
    maskfree = bool((mask == 1).all())
    in_maps = []
    # x^T -> [EP, 128, 2, S] fp8 pair tiles (e = 256j + 128r + p)
    x8b = [
        np.ascontiguousarray(
            x[b].T.reshape(EP, 2, PART, S).transpose(0, 2, 1, 3).astype(f8)
        )
        for b in range(B)
    ]
    if not maskfree:
        maskbias = (mask == 0).astype(np.float32) * NEG  # [B, S]
        mbTb = [
            np.ascontiguousarray(maskbias[b].reshape(KT, PART).T) for b in range(B)
        ]
        mb8 = np.clip(
            maskbias.astype(np.float64) * EXA8 + EXB8, -3.0e38, 3.0e38
        ).astype(np.float32)
        mbATb = [np.ascontiguousarray(mb8[b].reshape(KT, PART).T) for b in range(B)]
    slabs = {}
    for g in range(G):
        sl = slice(g * EG, (g + 1) * EG)
        wqT = np.asarray(Wq, np.float32)[sl].T  # [E, EG]
        wkT = np.asarray(Wk, np.float32)[sl].T
        wvT = np.asarray(Wv, np.float32)[sl].T
        # [MT, EP, 128, 2, 128]: wq8[dp, j, p, r, m] = wqT[256j+128r+p, 128dp+m]
        wq8 = np.ascontiguousarray(
            wqT.reshape(EP, 2, PART, MT, PART).transpose(3, 0, 2, 1, 4).astype(f8)
        )
        wk8 = np.ascontiguousarray(
            wkT.reshape(EP, 2, PART, MT, PART).transpose(3, 0, 2, 1, 4).astype(f8)
        )
        wv8 = np.ascontiguousarray(
            wvT.reshape(EP, 2, PART, EG).transpose(0, 2, 1, 3).astype(f8)
        )
        bq_t = np.ascontiguousarray(
            np.asarray(bq, np.float32)[sl].reshape(MT, PART).T
        )
        bk_t = np.ascontiguousarray(
            np.asarray(bk, np.float32)[sl].reshape(MT, PART).T
        )
        slabs[g] = (wq8, wk8, wv8, bq_t, bk_t)
    for c in range(NCORES):
        b, g = c // G, c % G
        wq8, wk8, wv8, bq_t, bk_t = slabs[g]
        m = {
            "x8T": x8b[b],
            "wq8T": wq8,
            "wk8T": wk8,
            "wv8T": wv8,
            "bqT": bq_t,
            "bkT": bk_t,
        }
        if not maskfree:
            m["mbT"] = mbTb[b]
            m["mbAT"] = mbATb[b]
        in_maps.append(m)
    return in_maps, maskfree


def host_tail(mean_attn, text_array, bv, Wo, bo, W1, b1, W2, b2):
    """Exact tail on [B, E]: out_proj (after the mean), normalize, sub, MLP."""
    out = mean_attn + np.asarray(bv, np.float32)[None, :]
    out = out @ np.asarray(Wo, np.float32).T + np.asarray(bo, np.float32)
    out = out / np.linalg.norm(out, axis=-1, keepdims=True)
    out = out - np.asarray(text_array, np.float32)
    h = np.maximum(out @ np.asarray(W1, np.float32).T + np.asarray(b1, np.float32), 0.0)
    return np.tanh(h @ np.asarray(W2, np.float32).T + np.asarray(b2, np.float32))


def kernel(
    x, mask, text_array, Wq, bq, Wk, bk, Wv, bv, Wo, bo, W1, b1, W2, b2
):
    from concourse.bass_utils import run_bass_kernel_spmd

    in_maps, maskfree = make_in_maps(x, mask, Wq, bq, Wk, bk, Wv)
    nc = get_nc(maskfree)
    out = run_bass_kernel_spmd(nc, in_maps, core_ids=list(range(NCORES)))
    mean_attn = np.zeros((B, E), np.float32)
    for c in range(NCORES):
        b, g = c // G, c % G
        pz = np.asarray(out.results[c]["resP"][0], np.float32)  # [HG, 65, S]
        r = np.einsum("hdq,hq->hd", pz[:, 0:64, :], 1.0 / pz[:, 64, :])
        mean_attn[b, g * EG : (g + 1) * EG] = r.reshape(EG) / S
    return host_tail(mean_attn, text_array, bv, Wo, bo, W1, b1, W2, b2).astype(
        np.float32
    )


# revision 30
# speedup vs baseline: 1.0059x; 1.0059x over previous
"""Trainium2 Bass kernel for nn_MultiHeadAttention_446676599023.

Strategy (8 NeuronCores, SPMD, no collectives):
  core c -> batch b = c//2, head-group g = c%2 (heads 8g..8g+7, E-dims 512g..512g+512).

Math: reference computes attn_out = softmax(QK^T/sqrt(D)) @ V per head, projects with
Wo, takes mean over sequence, normalizes, subtracts text_array, then a tiny MLP.
mean_S commutes with the output projection, so each core only needs
  r_h[d] = sum_q softmax_row(q) @ V_h  summed over q   (shape [64] per head)
and the whole Wo/normalize/MLP tail runs on host on a [4,1024] tensor (exact algebra,
negligible FLOPs). Device work per core:
  - x^T and all projection weights ship as fp8 e4m3; Q/K/V projections run as
    fp8 DoubleRow matmuls (256-wide contraction per instruction -> half the
    accumulation passes of bf16); x^T and the weights spread across the
    sync/gpsimd DMA queues (~27GB/s each).
  - Q^T,K^T land in SBUF as fp8 e4m3 [d-part, seq-free] (ScalarE converts,
    bias fused); V as fp8 [seq-part, d-free] with a ones column per head
    (66-stride) so the EV matmul also emits the denominator Z as row 64.
    Keeping every matmul operand fp8 matters: the PE pays ~130ns per
    bf16<->fp8 datapath-mode switch, so the stream is kept single-mode.
  - scores^T[k,q] = plain fp8 matmul (contraction d=64) written as
    [128,2,512] f32 PSUM pairs spanning two banks, single-shot per half.
  - E = exp(scores/8): one paired op per [128,2,512] tile, split ~115:141
    between ScalarE (table exp -> f8e5) and DVE (Schraudolph int8 affine,
    bitcast f8e5, ~2-4% rel err, attenuated to ~1e-3 by the tail). With an
    all-ones mask the exp ops take no bias vector, enabling the paired form;
    a general-mask fallback path uses per-kt ops with mask-bias vectors.
    (Do NOT fold extra bias into EXB8: shifting the affine moves the int8
    NaN window from -10.4 sigma up into the score distribution's tail.)
  - P^T[d,q] (+ Z row) = DoubleRow matmul(lhsT=V_aug[k,2,65], rhs=E^T[k,2,q])
    accumulated over the 8 key-tile pairs. Heads run sequentially so PSUM
    fits exactly (3 score pairs 12KB/part + pacc 2KB + proj 2KB), and each
    head's EV burst + drain is deferred into the next head's window so the
    PE never barriers on the slowest exp.
  - finalize per (head, q-chunk): one drain to SBUF bf16, DMA ships it out.
Host does r[d] = sum_q P[d,q]/Z[q] (0.014%% of total FLOPs), then /S and the exact
Wo/normalize/MLP tail.
"""

import math
import os
import sys

import numpy as np

for _p in ("/opt/trn_rl_repo",):
    if _p not in sys.path and os.path.isdir(_p):
        sys.path.append(_p)

B, S, E, H = 4, 2048, 1024, 16
D = E // H            # 64 head dim
G = 2                 # head groups (tensor-parallel factor)
EG = E // G           # 512 dims per group
HG = H // G           # 8 heads per group
NCORES = 8
PART = 128
EP = 4                # fp8 contraction pair-tiles (256 e-dims each)
KT = S // PART        # 16 key tiles
JP = KT // 2          # 8 key-tile pairs
MT = EG // PART       # 4 head-pairs per group
QC = 4                # q chunks
QW = S // QC          # 512
NEG = -1.0e30

# Schraudolph fast-exp in fp8-e5m2 bit space: bitcast(int8(x*EXA8 + EXB8))
# ~= exp(x) as e5m2. For raw scores s in [-80, 80] the affine lands in
# [3, 117] -- always positive, never in the NaN encodings (>=124), and the
# masked bias saturates the int8 convert to -128 = -0.0.
EXA8 = 5.770780163555851   # 2^2 / ln(2)
EXB8 = 59.825              # 15 * 4 - 366393/2^21 (min max-rel-err bias)

_CACHE: dict = {}


def _build(maskfree: bool = True, repeat: int = 1):
    """Build the Bacc module (one SPMD program, same on all 8 cores)."""
    import concourse.bacc as bacc
    import concourse.mybir as mybir
    import concourse.tile as tile
    from contextlib import ExitStack

    f32 = mybir.dt.float32
    bf16 = mybir.dt.bfloat16
    f8 = mybir.dt.float8e4   # x / weights / V tiles (values ~N(0,1))
    f8e = mybir.dt.float8e5  # e tiles (exp up to e^9.8 needs e5m2 range)
    i8 = mybir.dt.int8
    PM = mybir.MatmulPerfMode
    AF = mybir.ActivationFunctionType
    ALU = mybir.AluOpType

    nc = bacc.Bacc("TRN2", target_bir_lowering=False, debug=False)
    x8T = nc.dram_tensor("x8T", [EP, PART, 2, S], f8, kind="ExternalInput").ap()
    wq8T = nc.dram_tensor(
        "wq8T", [MT, EP, PART, 2, PART], f8, kind="ExternalInput"
    ).ap()
    wk8T = nc.dram_tensor(
        "wk8T", [MT, EP, PART, 2, PART], f8, kind="ExternalInput"
    ).ap()
    wv8T = nc.dram_tensor("wv8T", [EP, PART, 2, EG], f8, kind="ExternalInput").ap()
    bqT = nc.dram_tensor("bqT", [PART, MT], f32, kind="ExternalInput").ap()
    bkT = nc.dram_tensor("bkT", [PART, MT], f32, kind="ExternalInput").ap()
    if not maskfree:
        mbT = nc.dram_tensor("mbT", [PART, KT], f32, kind="ExternalInput").ap()
        mbAT = nc.dram_tensor("mbAT", [PART, KT], f32, kind="ExternalInput").ap()
    resP = nc.dram_tensor(
        "resP", [repeat, HG, 65, S], bf16, kind="ExternalOutput"
    ).ap()

    with tile.TileContext(nc) as tc, ExitStack() as ctx:
        const_p = ctx.enter_context(tc.tile_pool(name="const", bufs=1))
        x8_p = ctx.enter_context(tc.tile_pool(name="x8", bufs=EP))
        wv_p = ctx.enter_context(tc.tile_pool(name="wv", bufs=EP))
        wqk_p = ctx.enter_context(tc.tile_pool(name="wqk", bufs=2 * EP * 2))
        qt_p = ctx.enter_context(tc.tile_pool(name="qt", bufs=2))
        kt_p = ctx.enter_context(tc.tile_pool(name="kt", bufs=2))
        v_p = ctx.enter_context(tc.tile_pool(name="v", bufs=JP))
        ea_p = ctx.enter_context(tc.tile_pool(name="ea", bufs=JP))
        eb_p = ctx.enter_context(tc.tile_pool(name="eb", bufs=JP))
        psb_p = ctx.enter_context(tc.tile_pool(name="psb", bufs=4))
        # PSUM (16KB/partition = 8 banks): 3 score pair-tiles [128,2,512] f32
        # (4KB each) + pacc [65,512] (2KB) + proj accumulator [128,512] (2KB).
        scp_ps = ctx.enter_context(tc.tile_pool(name="scps", bufs=3, space="PSUM"))
        p_ps = ctx.enter_context(tc.tile_pool(name="pps", bufs=1, space="PSUM"))
        qk_ps = ctx.enter_context(tc.tile_pool(name="qkps", bufs=1, space="PSUM"))

        # DMA queue rotation for outputs (gpsimd is idle after the prologue;
        # sync carries the weight stream early on)
        out_qs = (nc.gpsimd, nc.sync)

        for rep in range(repeat):
            # ---- input DMAs: x8 first (it gates the first projection mms),
            # then p=0 weights, spread across queues so the transfers
            # parallelize over DMA engines ----
            x8 = []
            x8_qs = (nc.gpsimd, nc.sync, nc.gpsimd, nc.sync)
            for j in range(EP):
                t = x8_p.tile([PART, 2, S], f8, tag="x8")
                x8_qs[j].dma_start(t[:], x8T[j])
                x8.append(t)
            bq = const_p.tile([PART, MT], f32, tag="bq")
            nc.sync.dma_start(bq[:], bqT[:])
            bk = const_p.tile([PART, MT], f32, tag="bk")
            nc.sync.dma_start(bk[:], bkT[:])
            if not maskfree:
                mb = const_p.tile([PART, KT], f32, tag="mb")
                nc.sync.dma_start(mb[:], mbT[:])
                mb8 = const_p.tile([PART, KT], f32, tag="mb8")
                nc.sync.dma_start(mb8[:], mbAT[:])
            wk8 = {}
            wq8 = {}

            def emit_wqk_dma(p):
                for wT, store in ((wk8T, wk8), (wq8T, wq8)):
                    tiles = []
                    for j in range(EP):
                        t = wqk_p.tile([PART, 2, PART], f8, tag="wqk")
                        nc.sync.dma_start(t[:], wT[p, j])
                        tiles.append(t)
                    store[p] = tiles

            emit_wqk_dma(0)
            wv8 = []
            for j in range(EP):
                t = wv_p.tile([PART, 2, EG], f8, tag="wv")
                nc.gpsimd.dma_start(t[:], wv8T[j])
                wv8.append(t)
            for p in range(1, MT):
                emit_wqk_dma(p)

            # ---- emission helpers ----
            v_sb = [None] * JP  # fp8 pair tiles [128, 2, HG*66]

            def emit_vproj(ks):
                j2, sub = ks // 2, ks % 2
                if sub == 0:
                    v_sb[j2] = v_p.tile(
                        [PART, 2, HG * 66], f8, tag="v", name=f"vp{j2}"
                    )
                v3 = v_sb[j2][:][:, sub, :].rearrange("p (h c) -> p h c", c=66)
                nc.vector.memset(v3[:, :, 64:66], 1.0)
                ps = qk_ps.tile([PART, EG], f32, tag="qkp", name="vps")
                for j in range(EP):
                    nc.tensor.matmul(
                        ps[:],
                        lhsT=x8[j][:][:, :, ks * PART : (ks + 1) * PART],
                        rhs=wv8[j][:],
                        start=(j == 0),
                        stop=(j == EP - 1),
                        perf_mode=PM.DoubleRow,
                    )
                # DVE cannot narrow f32->fp8; ScalarE can
                nc.scalar.copy(
                    v3[:, :, 0:64],
                    ps[:].rearrange("p (h c) -> p h c", c=64),
                )

            qkmats = {}

            def emit_qkproj_group(p, which, qc2):
                # which: 0 = K, 1 = Q; qc2: chunk of S (columns of K^T/Q^T)
                if (p, 0) not in qkmats and which == 0 and qc2 == 0:
                    qkmats[(p, 0)] = kt_p.tile(
                        [PART, S], f8, tag="kt", name=f"kt{p}"
                    )
                    qkmats[(p, 1)] = qt_p.tile(
                        [PART, S], f8, tag="qt", name=f"qt{p}"
                    )
                dst = qkmats[(p, which)]
                bias = (bk, bq)[which]
                wtiles = (wk8, wq8)[which][p]
                ps = qk_ps.tile([PART, QW], f32, tag="qkp", name="qkps")
                for j in range(EP):
                    nc.tensor.matmul(
                        ps[:],
                        lhsT=wtiles[j][:],
                        rhs=x8[j][:][:, :, qc2 * QW : (qc2 + 1) * QW],
                        start=(j == 0),
                        stop=(j == EP - 1),
                        perf_mode=PM.DoubleRow,
                    )
                # fp8 narrowing must go through ScalarE; bias rides along
                nc.scalar.activation(
                    dst[:, qc2 * QW : (qc2 + 1) * QW],
                    ps[:],
                    AF.Identity,
                    bias=bias[:, p : p + 1],
                )

            # ---- prologue: just enough for (p0, qc0, h0) to start, then V
            # and the rest stream in as fillers inside the attention loop ----
            for qc2 in range(QC):
                emit_qkproj_group(0, 0, qc2)   # K p0, full S
            emit_qkproj_group(0, 1, 0)         # Q p0, qc0
            emit_vproj(0)
            emit_vproj(1)

            # filler schedule: list of emit-closures per (p, qc, hl)
            def fillers_for(p, qc, hl):
                if p == 0:
                    if qc == 0 and hl == 0:
                        return [(emit_vproj, ks) for ks in range(2, 16)]
                    if qc == 0 and hl == 1:
                        return [(emit_qkproj_group, 0, 1, 1)]
                    if qc == 1:
                        return [(emit_qkproj_group, 0, 1, 2 + hl)]
                    # qc 2..3: next head-pair's K (4 chunks) + Q (4 chunks)
                    i0 = ((qc - 2) * 2 + hl) * 2
                    out = []
                    for i in (i0, i0 + 1):
                        which, c = (0, i) if i < 4 else (1, i - 4)
                        out.append((emit_qkproj_group, 1, which, c))
                    return out
                if p < MT - 1:
                    i = (qc * 2 + hl)
                    which, c = (0, i) if i < 4 else (1, i - 4)
                    return [(emit_qkproj_group, p + 1, which, c)]
                return []

            # ---- attention: heads sequential, kt pairs with 2-ahead
            # score pipelining; exp split ~115:141 ScalarE:DVE. Each head's
            # EV burst + drain is deferred into the NEXT head's window (after
            # its first scores) so the PE never barriers on the slowest exp.
            gidx = 0
            out_qi = 0
            pending = []

            def flush_pending():
                nonlocal out_qi
                if not pending:
                    return
                evs2, hcol2, p2, hl2, qc2 = pending.pop()
                pacc = p_ps.tile([65, QW], f32, tag="pp", name="pacc")
                for i, (jp2, et2, ia2) in enumerate(evs2):
                    nc.tensor.matmul(
                        pacc[:],
                        lhsT=v_sb[jp2][:][:, 0:2, hcol2 : hcol2 + 65],
                        rhs=et2[:] if ia2 else et2[:].bitcast(f8e),
                        start=(i == 0),
                        stop=(i == JP - 1),
                        perf_mode=PM.DoubleRow,
                    )
                psb = psb_p.tile([65, QW], bf16, tag="psb")
                nc.scalar.copy(psb[:], pacc[:])
                out_qs[out_qi % len(out_qs)].dma_start(
                    resP[rep, 2 * p2 + hl2, :, qc2 * QW : (qc2 + 1) * QW],
                    psb[:],
                )
                out_qi += 1

            for p in range(MT):
                for qc in range(QC):
                    for hl in (0, 1):
                        kt_m = qkmats[(p, 0)]
                        qt_m = qkmats[(p, 1)]
                        fills = list(fillers_for(p, qc, hl))
                        r0 = hl * 64
                        hcol = 66 * (2 * p + hl)
                        scp = [None] * JP

                        def emit_scores(jp):
                            t = scp_ps.tile(
                                [PART, 2, QW], f32, tag="sc", name=f"sc{jp}"
                            )
                            for half in (0, 1):
                                kt = 2 * jp + half
                                nc.tensor.matmul(
                                    t[:][:, half, :],
                                    lhsT=kt_m[
                                        r0 : r0 + 64, kt * PART : (kt + 1) * PART
                                    ],
                                    rhs=qt_m[r0 : r0 + 64, qc * QW : (qc + 1) * QW],
                                )
                            scp[jp] = t

                        emit_scores(0)
                        emit_scores(1)
                        flush_pending()
                        evs = []  # (jp, e-tile, is_act) -- EVs issued in the
                        # next window via flush_pending
                        for jp in range(JP):
                            if jp + 2 < JP:
                                emit_scores(jp + 2)
                            if fills:
                                # spread fillers evenly over remaining slots
                                k = -(-len(fills) // (JP - jp))
                                for _ in range(min(k, len(fills))):
                                    f = fills.pop(0)
                                    f[0](*f[1:])
                            use_act = ((gidx * 115) % 256) < 115
                            gidx += 1
                            if use_act:
                                et = ea_p.tile([PART, 2, QW], f8e, tag="et")
                                if maskfree:
                                    nc.scalar.activation(
                                        et[:],
                                        scp[jp][:],
                                        AF.Exp,
                                        scale=1.0 / math.sqrt(D),
                                    )
                                else:
                                    for half in (0, 1):
                                        kt = 2 * jp + half
                                        nc.scalar.activation(
                                            et[:][:, half, :],
                                            scp[jp][:][:, half, :],
                                            AF.Exp,
                                            bias=mb[:, kt : kt + 1],
                                            scale=1.0 / math.sqrt(D),
                                        )
                            else:
                                # Schraudolph e5m2 (safe affine range), same
                                # e^-2 bias folded into the constant
                                et = eb_p.tile([PART, 2, QW], i8, tag="ei")
                                if maskfree:
                                    nc.vector.tensor_scalar(
                                        et[:],
                                        scp[jp][:],
                                        EXA8 / 8.0,
                                        EXB8,
                                        ALU.mult,
                                        ALU.add,
                                    )
                                else:
                                    for half in (0, 1):
                                        kt = 2 * jp + half
                                        nc.vector.tensor_scalar(
                                            et[:][:, half, :],
                                            scp[jp][:][:, half, :],
                                            EXA8 / 8.0,
                                            mb8[:, kt : kt + 1],
                                            ALU.mult,
                                            ALU.add,
                                        )
                            evs.append((jp, et, use_act))
                        pending.append((evs, hcol, p, hl, qc))
            flush_pending()

    nc.compile()
    return nc


def get_nc(maskfree: bool = True, repeat: int = 1):
    key = ("nc", maskfree, repeat)
    if key not in _CACHE:
        _CACHE[key] = _build(maskfree, repeat)
    return _CACHE[key]


def make_in_maps(x, mask, Wq, bq, Wk, bk, Wv):
    """Per-core input dict (core c -> batch c//2, head-group c%2)."""
    import ml_dtypes

    f8 = ml_dtypes.float8_e4m3
    x = np.asarray(x, np.float32)
    mask = np.asarray(mask)
    maskfree = bool((mask == 1).all())
    in_maps = []
    # x^T -> [EP, 128, 2, S] fp8 pair tiles (e = 256j + 128r + p)
    x8b = [
        np.ascontiguousarray(
            x[b].T.reshape(EP, 2, PART, S).transpose(0, 2, 1, 3).astype(f8)
        )
        for b in range(B)
    ]
    if not maskfree:
        maskbias = (mask == 0).astype(np.float32) * NEG  # [B, S]
        mbTb = [
            np.ascontiguousarray(maskbias[b].reshape(KT, PART).T) for b in range(B)
        ]
        mb8 = np.clip(
            maskbias.astype(np.float64) * EXA8 + EXB8, -3.0e38, 3.0e38
        ).astype(np.float32)
        mbATb = [np.ascontiguousarray(mb8[b].reshape(KT, PART).T) for b in range(B)]
    slabs = {}
    for g in range(G):
        sl = slice(g * EG, (g + 1) * EG)
        wqT = np.asarray(Wq, np.float32)[sl].T  # [E, EG]
        wkT = np.asarray(Wk, np.float32)[sl].T
        wvT = np.asarray(Wv, np.float32)[sl].T
        # [MT, EP, 128, 2, 128]: wq8[dp, j, p, r, m] = wqT[256j+128r+p, 128dp+m]
        wq8 = np.ascontiguousarray(
            wqT.reshape(EP, 2, PART, MT, PART).transpose(3, 0, 2, 1, 4).astype(f8)
        )
        wk8 = np.ascontiguousarray(
            wkT.reshape(EP, 2, PART, MT, PART).transpose(3, 0, 2, 1, 4).astype(f8)
        )
        wv8 = np.ascontiguousarray(
            wvT.reshape(EP, 2, PART, EG).transpose(0, 2, 1, 3).astype(f8)
        )
        bq_t = np.ascontiguousarray(
            np.asarray(bq, np.float32)[sl].reshape(MT, PART).T
        )
        bk_t = np.ascontiguousarray(
            np.asarray(bk, np.float32)[sl].reshape(MT, PART).T
        )
        slabs[g] = (wq8, wk8, wv8, bq_t, bk_t)
    for c in range(NCORES):
        b, g = c // G, c % G
        wq8, wk8, wv8, bq_t, bk_t = slabs[g]
        m = {
            "x8T": x8b[b],
            "wq8T": wq8,
            "wk8T": wk8,
            "wv8T": wv8,
            "bqT": bq_t,
            "bkT": bk_t,
        }
        if not maskfree:
            m["mbT"] = mbTb[b]
            m["mbAT"] = mbATb[b]
        in_maps.append(m)
    return in_maps, maskfree


def host_tail(mean_attn, text_array, bv, Wo, bo, W1, b1, W2, b2):
    """Exact tail on [B, E]: out_proj (after the mean), normalize, sub, MLP."""
    out = mean_attn + np.asarray(bv, np.float32)[None, :]
    out = out @ np.asarray(Wo, np.float32).T + np.asarray(bo, np.float32)
    out = out / np.linalg.norm(out, axis=-1, keepdims=True)
    out = out - np.asarray(text_array, np.float32)
    h = np.maximum(out @ np.asarray(W1, np.float32).T + np.asarray(b1, np.float32), 0.0)
    return np.tanh(h @ np.asarray(W2, np.float32).T + np.asarray(b2, np.float32))


def kernel(
    x, mask, text_array, Wq, bq, Wk, bk, Wv, bv, Wo, bo, W1, b1, W2, b2
):
    from concourse.bass_utils import run_bass_kernel_spmd

    in_maps, maskfree = make_in_maps(x, mask, Wq, bq, Wk, bk, Wv)
    nc = get_nc(maskfree)
    out = run_bass_kernel_spmd(nc, in_maps, core_ids=list(range(NCORES)))
    mean_attn = np.zeros((B, E), np.float32)
    for c in range(NCORES):
        b, g = c // G, c % G
        pz = np.asarray(out.results[c]["resP"][0], np.float32)  # [HG, 65, S]
        r = np.einsum("hdq,hq->hd", pz[:, 0:64, :], 1.0 / pz[:, 64, :])
        mean_attn[b, g * EG : (g + 1) * EG] = r.reshape(EG) / S
    return host_tail(mean_attn, text_array, bv, Wo, bo, W1, b1, W2, b2).astype(
        np.float32
    )
